# revision 14
# baseline (speedup 1.0000x reference)
"""Trainium2 Bass kernel: CustomFlashAttention (B=1, S=2048, D=2048, H=16, Hd=128).

Sharding (Megatron tensor-parallel over heads, 8 NeuronCores):
  - each core owns 2 heads (256 feature dims)
  - w_q/w_k/w_v column-parallel (pre-transposed + sliced on host)
  - w_o row-parallel; cores produce partial outputs (fp16), host sums them

Device layout: activations are feature-major ([feat, seq]) so every matmul's
contraction dim lands on SBUF partitions with zero on-device transposes.
Scores are computed transposed sT[k, q] = K Q^T; softmax runs without
max-subtraction (scores ~ N(0,1)); exp'd fp16 tiles feed P^T into the PV
matmul.

Softmax denominators: exp tiles are accumulated elementwise on the vector
engine (fp16 2x DVE mode) into one tile per block, partition-reduced with a
single ones-matmul, and inverted with the fast approximate reciprocal.

Schedule: one flat software-pipelined stream over all attention blocks
(q-chunks 512,512,512,256,256 x 2 heads) x 16 k-tiles. Score matmuls run 3
k-tiles ahead of the PV matmuls and flow across block boundaries, so the
scalar engine's exp latency never stalls the in-order PE. The q projection
of the next chunk and the output projection of the previous chunk are paced
into the stream as filler matmuls; the last chunk is 256 wide so only a
half-width output projection remains past the final softmax. Output is
stored fp16.
"""

import sys
from contextlib import ExitStack

import numpy as np

if "/opt/trn_rl_repo" not in sys.path:
    sys.path.insert(0, "/opt/trn_rl_repo")

import concourse.bass as bass  # noqa: F401
import concourse.tile as tile
from concourse import bacc, mybir
from concourse.bass_utils import run_bass_kernel_spmd

P = 128                      # SBUF partitions
S = 2048                     # sequence length
D = 2048                     # hidden dim
H = 16                       # heads
HD = 128                     # head dim
NCORES = 8
HPC = H // NCORES            # heads per core = 2
HDC = HPC * HD               # feature dims per core = 256
DT = D // P                  # 16 contraction tiles
NCH = 4                      # phase-1 seq chunks
CH = S // NCH                # 512
KT = S // P                  # 16 key tiles
SCALE = 1.0 / float(np.sqrt(HD))

# attention q-chunks: (start, width)
ACH = [(0, 512), (512, 512), (1024, 512), (1536, 256), (1792, 256)]

f32 = mybir.dt.float32
f16 = mybir.dt.float16

_CACHE = {}
LAST_RESULT = None


def _build_nc():
    nc = bacc.Bacc("TRN2", target_bir_lowering=False, debug=False, num_devices=NCORES)

    xT = nc.dram_tensor("xT", [D, S], f16, kind="ExternalInput").ap()
    wqT = nc.dram_tensor("wqT", [D, HDC], f16, kind="ExternalInput").ap()
    wkT = nc.dram_tensor("wkT", [D, HDC], f16, kind="ExternalInput").ap()
    wvT = nc.dram_tensor("wvT", [D, HDC], f16, kind="ExternalInput").ap()
    woT = nc.dram_tensor("woT", [HDC, D], f16, kind="ExternalInput").ap()
    outT = nc.dram_tensor("outT", [D, S], f16, kind="ExternalOutput").ap()

    xT_r = xT.rearrange("(dt p) s -> dt p s", p=P)       # [16, 128, 2048]
    out_r = outT.rearrange("(ot p) s -> ot p s", p=P)    # [16, 128, 2048]

    def csl(c):
        return slice(c * CH, (c + 1) * CH)

    def asl(ci):
        st, w = ACH[ci]
        return slice(st, st + w)

    with ExitStack() as ctx:
        tc = ctx.enter_context(tile.TileContext(nc))

        singles = ctx.enter_context(tc.tile_pool(name="singles", bufs=1))
        ptpool = ctx.enter_context(tc.tile_pool(name="pt", bufs=5))
        dapool = ctx.enter_context(tc.tile_pool(name="da", bufs=2))
        rspool = ctx.enter_context(tc.tile_pool(name="rs", bufs=2))
        obpool = ctx.enter_context(tc.tile_pool(name="ob", bufs=8))
        # Persistent SBUF tensors
        x_sb = singles.tile([P, DT, S], f16, tag="x")
        wq_sb = singles.tile([P, DT, HDC], f16, tag="wq")
        wk_sb = singles.tile([P, DT, HDC], f16, tag="wk")
        wv_sb = singles.tile([P, DT, HDC], f16, tag="wv")
        wo_sb = singles.tile([P, HDC // P, D], f16, tag="wo")
        qT_sb = singles.tile([P, HPC, S], f16, tag="qT")
        kT_sb = singles.tile([P, HPC, S], f16, tag="kT")
        v_sb = singles.tile([P, KT, HDC], f16, tag="v")
        oT_sb = singles.tile([P, HPC, S], f16, tag="oT")
        ones = singles.tile([P, P], f16, tag="ones")

        nc.vector.memset(ones, 1.0)

        p1_ctx = ExitStack()
        k_ps = p1_ctx.enter_context(tc.tile_pool(name="kps", bufs=2, space="PSUM"))
        q_ps = p1_ctx.enter_context(tc.tile_pool(name="qps", bufs=2, space="PSUM"))
        v_ps = p1_ctx.enter_context(tc.tile_pool(name="vps", bufs=4, space="PSUM"))

        # ---------------- DMA schedule ----------------
        # Ordered by first use: (wk, wv, x chunk0) d-groups, then x chunk1,
        # then wq (q pass runs after chunk-1 k/v), wo, then x chunks 2-3.
        # Even-d tiles + wk + wo ride sync; odd-d + wv + wq ride gpsimd.
        wk_r = wkT.rearrange("(dt p) h -> p dt h", p=P)
        wv_r = wvT.rearrange("(dt p) h -> p dt h", p=P)
        wq_r = wqT.rearrange("(dt p) h -> p dt h", p=P)
        xT_p = xT.rearrange("(dt p) s -> p dt s", p=P)   # partition-first view
        # chunk 0 + weights: fine-grained, interleaved in consumption order
        for q4 in range(4):
            dsl = slice(q4 * 4, (q4 + 1) * 4)
            nc.sync.dma_start(out=wk_sb[:, dsl, :], in_=wk_r[:, dsl, :])
            nc.gpsimd.dma_start(out=wv_sb[:, dsl, :], in_=wv_r[:, dsl, :])
            for d in range(q4 * 4, (q4 + 1) * 4):
                q = nc.sync if d % 2 == 0 else nc.gpsimd
                q.dma_start(out=x_sb[:, d, 0:CH], in_=xT_r[d][:, 0:CH])
        # later chunks: one medium batch per (parity, chunk)
        ev = slice(0, DT, 2)
        od = slice(1, DT, 2)
        nc.sync.dma_start(out=x_sb[:, ev, CH:2 * CH], in_=xT_p[:, ev, CH:2 * CH])
        nc.gpsimd.dma_start(out=x_sb[:, od, CH:2 * CH], in_=xT_p[:, od, CH:2 * CH])
        nc.gpsimd.dma_start(out=wq_sb, in_=wq_r)
        nc.sync.dma_start(out=wo_sb, in_=woT.rearrange("(it p) o -> p it o", p=P))
        nc.sync.dma_start(out=x_sb[:, ev, 2 * CH:3 * CH], in_=xT_p[:, ev, 2 * CH:3 * CH])
        nc.gpsimd.dma_start(out=x_sb[:, od, 2 * CH:3 * CH], in_=xT_p[:, od, 2 * CH:3 * CH])
        nc.sync.dma_start(out=x_sb[:, ev, 3 * CH:S], in_=xT_p[:, ev, 3 * CH:S])
        nc.gpsimd.dma_start(out=x_sb[:, od, 3 * CH:S], in_=xT_p[:, od, 3 * CH:S])

        # ---------------- Phase 1: k/v (+q chunk0) projections --------
        def kv_chunk(c):
            pk = [k_ps.tile([P, CH], f32, tag="pk", name=f"pk{c}_{i}") for i in range(HPC)]
            pv = [v_ps.tile([P, HDC], f32, tag="pv", name=f"pv{c}_{i}") for i in range(4)]
            for d in range(DT):
                first, last = (d == 0), (d == DT - 1)
                # long/short interleave so short-matmul weight loads hide
                for h in range(HPC):
                    nc.tensor.matmul(
                        pk[h],
                        lhsT=wk_sb[:, d, h * HD:(h + 1) * HD],
                        rhs=x_sb[:, d, csl(c)], start=first, stop=last,
                    )
                    for st in (0, 1) if h == 0 else (2, 3):
                        nc.tensor.matmul(
                            pv[st],
                            lhsT=x_sb[:, d, c * CH + st * P:c * CH + (st + 1) * P],
                            rhs=wv_sb[:, d, :],
                            start=first, stop=last,
                        )
            for h in range(HPC):
                nc.vector.tensor_copy(kT_sb[:, h, csl(c)], pk[h])
            for st in range(4):
                nc.vector.tensor_copy(v_sb[:, c * 4 + st, :], pv[st])

        kv_chunk(0)
        kv_chunk(1)
        # chunk-0 q pass (here so the PE never waits on the later wq DMA)
        pq0 = [q_ps.tile([P, CH], f32, tag="pq", name=f"pq0_{i}") for i in range(HPC)]
        for d in range(DT):
            for h in range(HPC):
                nc.tensor.matmul(
                    pq0[h],
                    lhsT=wq_sb[:, d, h * HD:(h + 1) * HD],
                    rhs=x_sb[:, d, csl(0)], start=(d == 0), stop=(d == DT - 1),
                )
        for h in range(HPC):
            nc.vector.tensor_copy(qT_sb[:, h, csl(0)], pq0[h])
        kv_chunk(2)
        kv_chunk(3)

        p1_ctx.close()  # release phase-1 PSUM banks

        # ---------------- attention pools ----------------
        sc_ps = ctx.enter_context(tc.tile_pool(name="scps", bufs=4, space="PSUM"))
        o_ps = ctx.enter_context(tc.tile_pool(name="ops", bufs=2, space="PSUM"))
        ro_ps = ctx.enter_context(tc.tile_pool(name="rops", bufs=2, space="PSUM"))
        pool_tag = {id(sc_ps): "psc", id(o_ps): "po", id(ro_ps): "rout"}

        # ---------------- filler emission units ----------------
        def make_defq_units(ci, h):
            """16 single-matmul units projecting q for attn chunk ci."""
            sl = asl(ci)
            w = ACH[ci][1]
            pq = ro_ps.tile([P, w], f32, tag="rout", name=f"dpq{ci}_{h}")

            def unit(d):
                def emit():
                    nc.tensor.matmul(
                        pq,
                        lhsT=wq_sb[:, d, h * HD:(h + 1) * HD],
                        rhs=x_sb[:, d, sl],
                        start=(d == 0), stop=(d == DT - 1),
                    )
                    if d == DT - 1:
                        nc.vector.tensor_copy(qT_sb[:, h, sl], pq)
                return emit

            return [unit(d) for d in range(DT)]

        def make_ph3_units(ci, pools=None):
            sl = asl(ci)
            w = ACH[ci][1]

            def unit(ot):
                def emit():
                    pool = pools[ot % len(pools)] if pools else ro_ps
                    pout = pool.tile([P, w], f32, tag=pool_tag.get(id(pool), "rout"),
                                     name=f"pout{ci}_{ot}")
                    for di in range(HDC // P):
                        nc.tensor.matmul(
                            pout,
                            lhsT=wo_sb[:, di, ot * P:(ot + 1) * P],
                            rhs=oT_sb[:, di, sl],
                            start=(di == 0), stop=(di == HDC // P - 1),
                        )
                    ob = obpool.tile([P, w], f16, tag="ob", name=f"ob{ci}_{ot}")
                    if ot % 2 == 1:
                        nc.scalar.copy(ob, pout)
                    else:
                        nc.vector.tensor_copy(ob, pout)
                    q = nc.sync if ot % 2 == 0 else nc.gpsimd
                    q.dma_start(out=out_r[ot][:, sl], in_=ob)
                return emit

            return [unit(ot) for ot in range(DT)]

        # ---------------- Phase 2: flat pipelined attention ----------------
        blocks = [(ci, h) for ci in range(len(ACH)) for h in range(HPC)]
        steps = [(bi, g) for bi in range(len(blocks)) for g in range(KT)]
        LOOKAHEAD = 3

        st_pt = {}      # (bi, g) -> pt AP for the PV matmul
        st_pair = {}    # bi -> (psc_pair, pt_pair) in flight
        st_dacc = {}    # bi -> dacc tile
        st_po = {}      # bi -> PV psum tile

        def dacc_accum(bi, g, a, b=None):
            # denominator accumulation on DVE (fp16, 2x mode)
            w = ACH[blocks[bi][0]][1]
            if g <= 1:
                dacc = dapool.tile([P, w], f16, tag="dacc", name=f"dacc{bi}")
                st_dacc[bi] = dacc
                nc.vector.tensor_add(dacc, a, b)
            else:
                nc.vector.tensor_add(st_dacc[bi], st_dacc[bi], a)
                if b is not None:
                    nc.vector.tensor_add(st_dacc[bi], st_dacc[bi], b)

        def emit_S(bi, g):
            ci, h = blocks[bi]
            w = ACH[ci][1]
            if w == CH:
                psc = sc_ps.tile([P, w], f32, tag="psc", name=f"psc{bi}_{g}")
                nc.tensor.matmul(
                    psc,
                    lhsT=kT_sb[:, h, g * P:(g + 1) * P],
                    rhs=qT_sb[:, h, asl(ci)],
                    start=True, stop=True,
                )
                pt = ptpool.tile([P, w], f16, tag="pt", name=f"pt{bi}_{g}")
                nc.scalar.activation(
                    out=pt, in_=psc,
                    func=mybir.ActivationFunctionType.Exp, scale=SCALE,
                )
                st_pt[(bi, g)] = pt
                if g == 1:
                    dacc_accum(bi, g, st_pt[(bi, 0)], pt)
                elif g > 1:
                    dacc_accum(bi, g, pt)
            else:
                # narrow blocks: two k-tiles share one psum tile and one exp
                if g % 2 == 0:
                    psc = sc_ps.tile([P, 2, w], f32, tag="psc", name=f"psc{bi}_{g}")
                    pt = ptpool.tile([P, 2, w], f16, tag="pt", name=f"pt{bi}_{g}")
                    st_pair[bi] = (psc, pt)
                psc, pt = st_pair[bi]
                nc.tensor.matmul(
                    psc[:, g % 2, :],
                    lhsT=kT_sb[:, h, g * P:(g + 1) * P],
                    rhs=qT_sb[:, h, asl(ci)],
                    start=True, stop=True,
                )
                st_pt[(bi, g)] = pt[:, g % 2, :]
                if g % 2 == 1:
                    nc.scalar.activation(
                        out=pt, in_=psc,
                        func=mybir.ActivationFunctionType.Exp, scale=SCALE,
                    )
                    dacc_accum(bi, g, pt[:, 0, :], pt[:, 1, :])

        def emit_PV(bi, g):
            ci, h = blocks[bi]
            if g == 0:
                st_po[bi] = o_ps.tile([P, ACH[ci][1]], f32, tag="po", name=f"po{bi}")
            nc.tensor.matmul(
                st_po[bi],
                lhsT=v_sb[:, g, h * HD:(h + 1) * HD],
                rhs=st_pt.pop((bi, g)),
                start=(g == 0), stop=(g == KT - 1),
            )

        def emit_epilogue(bi):
            ci, h = blocks[bi]
            w = ACH[ci][1]
            pr = sc_ps.tile([P, w], f32, tag="psc", name=f"pr{bi}")
            nc.tensor.matmul(pr, lhsT=ones, rhs=st_dacc[bi], start=True, stop=True)
            rs = rspool.tile([P, w], f32, tag="rs", name=f"rs{bi}")
            nc.vector.reciprocal_approx_fast(out=rs, in_=pr)
            nc.vector.tensor_mul(oT_sb[:, h, asl(ci)], st_po[bi], rs)

        # per-block filler lists: next chunk's q projection, then previous
        # chunks' output projections (ph3(2) is split 5/3 so the short final
        # blocks have enough PE work to cover their exp latency)
        ph3u = {ci: make_ph3_units(ci) for ci in range(len(ACH) - 1)}
        fillers = []
        for bi, (ci, h) in enumerate(blocks):
            L = []
            if ci + 1 < len(ACH):
                L += make_defq_units(ci + 1, h)
            if ci >= 1:
                L += ph3u[ci - 1][h * 8:(h + 1) * 8]
            fillers.append(L)
        emitted = [0] * len(blocks)

        for t in range(LOOKAHEAD):
            emit_S(*steps[t])
        for t, (bi, g) in enumerate(steps):
            if g == 0 and bi > 0:
                emit_epilogue(bi - 1)
            L = fillers[bi]
            want = ((g + 1) * len(L) + KT - 1) // KT
            while emitted[bi] < min(want, len(L)):
                L[emitted[bi]]()
                emitted[bi] += 1
            emit_PV(bi, g)
            if t + LOOKAHEAD < len(steps):
                emit_S(*steps[t + LOOKAHEAD])
        emit_epilogue(len(blocks) - 1)

        # final (256-wide) output projection, rotated across all free PSUM
        # pools so the PE streams it back-to-back
        for u in make_ph3_units(len(ACH) - 1, pools=[sc_ps, ro_ps, sc_ps, o_ps]):
            u()

    nc.compile()
    return nc


def _get_nc():
    if "nc" not in _CACHE:
        _CACHE["nc"] = _build_nc()
    return _CACHE["nc"]


def make_in_maps(x, w_q, w_k, w_v, w_o):
    x = np.asarray(x, dtype=np.float32).reshape(S, D)
    w_q = np.asarray(w_q, dtype=np.float32)
    w_k = np.asarray(w_k, dtype=np.float32)
    w_v = np.asarray(w_v, dtype=np.float32)
    w_o = np.asarray(w_o, dtype=np.float32)
    xT = np.ascontiguousarray(x.T).astype(np.float16)
    in_maps = []
    for c in range(NCORES):
        hs = slice(c * HDC, (c + 1) * HDC)
        in_maps.append({
            "xT": xT,
            "wqT": np.ascontiguousarray(w_q[hs, :].T).astype(np.float16),
            "wkT": np.ascontiguousarray(w_k[hs, :].T).astype(np.float16),
            "wvT": np.ascontiguousarray(w_v[hs, :].T).astype(np.float16),
            "woT": np.ascontiguousarray(w_o[:, hs].T).astype(np.float16),
        })
    return in_maps


def kernel(x, w_q, w_k, w_v, w_o):
    global LAST_RESULT
    in_maps = make_in_maps(x, w_q, w_k, w_v, w_o)
    nc = _get_nc()
    res = run_bass_kernel_spmd(nc, in_maps, core_ids=list(range(NCORES)))
    LAST_RESULT = res
    acc = np.zeros((D, S), dtype=np.float32)
    for r in res.results:
        acc += r["outT"].astype(np.float32)
    return np.ascontiguousarray(acc.T).astype(np.float32).reshape(1, S, D)


# revision 19
# speedup vs baseline: 1.2667x; 1.2667x over previous
"""Trainium2 Bass kernel: CustomFlashAttention (B=1, S=2048, D=2048, H=16, Hd=128).

Sharding (Megatron tensor-parallel over heads, 8 NeuronCores):
  - each core owns 2 heads (256 feature dims)
  - w_q/w_k/w_v column-parallel (pre-transposed + sliced on host)
  - w_o row-parallel; cores produce partial outputs (fp16), host sums them

Device layout: activations are feature-major ([feat, seq]) so every matmul's
contraction dim lands on SBUF partitions with zero on-device transposes.
Scores are computed transposed sT[k, q] = K Q^T; softmax runs without
max-subtraction (scores ~ N(0,1)); exp'd fp16 tiles feed P^T into the PV
matmul.

Softmax denominators: exp tiles are accumulated elementwise on the vector
engine (fp16 2x DVE mode) into one tile per block, partition-reduced with a
single ones-matmul, and inverted with the fast approximate reciprocal.

Schedule: one flat software-pipelined stream over all attention blocks
(q-chunks 512,512,512,256,256 x 2 heads) x 16 k-tiles. Score matmuls run 3
k-tiles ahead of the PV matmuls and flow across block boundaries, so the
scalar engine's exp latency never stalls the in-order PE. The q projection
of the next chunk and the output projection of the previous chunk are paced
into the stream as filler matmuls; the last chunk is 256 wide so only a
half-width output projection remains past the final softmax. Output is
stored fp16.
"""

import sys
from contextlib import ExitStack

import numpy as np

if "/opt/trn_rl_repo" not in sys.path:
    sys.path.insert(0, "/opt/trn_rl_repo")

import concourse.bass as bass  # noqa: F401
import concourse.tile as tile
from concourse import bacc, mybir
from concourse.bass_utils import run_bass_kernel_spmd

P = 128                      # SBUF partitions
S = 2048                     # sequence length
D = 2048                     # hidden dim
H = 16                       # heads
HD = 128                     # head dim
NCORES = 8
HPC = H // NCORES            # heads per core = 2
HDC = HPC * HD               # feature dims per core = 256
DT = D // P                  # 16 contraction tiles
NCH = 4                      # phase-1 seq chunks
CH = S // NCH                # 512
KT = S // P                  # 16 key tiles
SCALE = 1.0 / float(np.sqrt(HD))

# attention q-chunks: (start, width)
ACH = [(0, 512), (512, 512), (1024, 512), (1536, 256), (1792, 256)]

f32 = mybir.dt.float32
f16 = mybir.dt.float16

_CACHE = {}
LAST_RESULT = None


def _build_nc():
    nc = bacc.Bacc("TRN2", target_bir_lowering=False, debug=False, num_devices=NCORES)

    xT = nc.dram_tensor("xT", [D, S], f16, kind="ExternalInput").ap()
    wqT = nc.dram_tensor("wqT", [D, HDC], f16, kind="ExternalInput").ap()
    wkT = nc.dram_tensor("wkT", [D, HDC], f16, kind="ExternalInput").ap()
    wvT = nc.dram_tensor("wvT", [D, HDC], f16, kind="ExternalInput").ap()
    woT = nc.dram_tensor("woT", [HDC, D], f16, kind="ExternalInput").ap()
    outT = nc.dram_tensor("outT", [D, S], f16, kind="ExternalOutput").ap()

    xT_r = xT.rearrange("(dt p) s -> dt p s", p=P)       # [16, 128, 2048]
    out_r = outT.rearrange("(ot p) s -> ot p s", p=P)    # [16, 128, 2048]

    def csl(c):
        return slice(c * CH, (c + 1) * CH)

    def asl(ci):
        st, w = ACH[ci]
        return slice(st, st + w)

    with ExitStack() as ctx:
        tc = ctx.enter_context(tile.TileContext(nc))

        singles = ctx.enter_context(tc.tile_pool(name="singles", bufs=1))
        ptpool = ctx.enter_context(tc.tile_pool(name="pt", bufs=5))
        dapool = ctx.enter_context(tc.tile_pool(name="da", bufs=2))
        rspool = ctx.enter_context(tc.tile_pool(name="rs", bufs=2))
        obpool = ctx.enter_context(tc.tile_pool(name="ob", bufs=8))
        # Persistent SBUF tensors
        x_sb = singles.tile([P, DT, S], f16, tag="x")
        wq_sb = singles.tile([P, DT, HDC], f16, tag="wq")
        wk_sb = singles.tile([P, DT, HDC], f16, tag="wk")
        wv_sb = singles.tile([P, DT, HDC], f16, tag="wv")
        wo_sb = singles.tile([P, HDC // P, D], f16, tag="wo")
        qT_sb = singles.tile([P, HPC, S], f16, tag="qT")
        kT_sb = singles.tile([P, HPC, S], f16, tag="kT")
        v_sb = singles.tile([P, KT, HDC], f16, tag="v")
        oT_sb = singles.tile([P, HPC, S], f16, tag="oT")
        ones = singles.tile([P, P], f16, tag="ones")

        nc.vector.memset(ones, 1.0)

        p1_ctx = ExitStack()
        k_ps = p1_ctx.enter_context(tc.tile_pool(name="kps", bufs=2, space="PSUM"))
        q_ps = p1_ctx.enter_context(tc.tile_pool(name="qps", bufs=2, space="PSUM"))
        v_ps = p1_ctx.enter_context(tc.tile_pool(name="vps", bufs=4, space="PSUM"))

        # ---------------- DMA schedule ----------------
        # Ordered by first use: (wk, wv, x chunk0) d-groups, then x chunk1,
        # then wq (q pass runs after chunk-1 k/v), wo, then x chunks 2-3.
        # Even-d tiles + wk + wo ride sync; odd-d + wv + wq ride gpsimd.
        wk_r = wkT.rearrange("(dt p) h -> p dt h", p=P)
        wv_r = wvT.rearrange("(dt p) h -> p dt h", p=P)
        wq_r = wqT.rearrange("(dt p) h -> p dt h", p=P)
        xT_p = xT.rearrange("(dt p) s -> p dt s", p=P)   # partition-first view
        # chunk 0 + weights: fine-grained, interleaved in consumption order;
        # wq quarters ride along so the q pass (right after chunk 0) is fed
        for q4 in range(4):
            dsl = slice(q4 * 4, (q4 + 1) * 4)
            nc.sync.dma_start(out=wk_sb[:, dsl, :], in_=wk_r[:, dsl, :])
            nc.gpsimd.dma_start(out=wv_sb[:, dsl, :], in_=wv_r[:, dsl, :])
            for d in range(q4 * 4, (q4 + 1) * 4):
                q = nc.sync if d % 2 == 0 else nc.gpsimd
                q.dma_start(out=x_sb[:, d, 0:CH], in_=xT_r[d][:, 0:CH])
            if q4 < 2:
                qsl = slice(q4 * 4, (q4 + 1) * 4)
                nc.gpsimd.dma_start(out=wq_sb[:, qsl, :], in_=wq_r[:, qsl, :])
        for q4 in range(2, 4):
            qsl = slice(q4 * 4, (q4 + 1) * 4)
            nc.gpsimd.dma_start(out=wq_sb[:, qsl, :], in_=wq_r[:, qsl, :])
        # later chunks: medium batches per (parity, chunk); c1 split finer
        ev1 = slice(0, DT // 2, 2)
        ev2 = slice(DT // 2, DT, 2)
        od1 = slice(1, DT // 2, 2)
        od2 = slice(DT // 2 + 1, DT, 2)
        ev = slice(0, DT, 2)
        od = slice(1, DT, 2)
        nc.sync.dma_start(out=x_sb[:, ev1, CH:2 * CH], in_=xT_p[:, ev1, CH:2 * CH])
        nc.gpsimd.dma_start(out=x_sb[:, od1, CH:2 * CH], in_=xT_p[:, od1, CH:2 * CH])
        nc.sync.dma_start(out=x_sb[:, ev2, CH:2 * CH], in_=xT_p[:, ev2, CH:2 * CH])
        nc.gpsimd.dma_start(out=x_sb[:, od2, CH:2 * CH], in_=xT_p[:, od2, CH:2 * CH])
        nc.sync.dma_start(out=x_sb[:, ev, 2 * CH:3 * CH], in_=xT_p[:, ev, 2 * CH:3 * CH])
        nc.gpsimd.dma_start(out=x_sb[:, od, 2 * CH:3 * CH], in_=xT_p[:, od, 2 * CH:3 * CH])
        nc.sync.dma_start(out=x_sb[:, ev, 3 * CH:S], in_=xT_p[:, ev, 3 * CH:S])
        nc.gpsimd.dma_start(out=x_sb[:, od, 3 * CH:S], in_=xT_p[:, od, 3 * CH:S])
        nc.sync.dma_start(out=wo_sb, in_=woT.rearrange("(it p) o -> p it o", p=P))

        # ---------------- Phase 1: k/v (+q chunk0) projections --------
        def kv_chunk(c):
            pk = [k_ps.tile([P, CH], f32, tag="pk", name=f"pk{c}_{i}") for i in range(HPC)]
            pv = [v_ps.tile([P, HDC], f32, tag="pv", name=f"pv{c}_{i}") for i in range(4)]
            for d in range(DT):
                first, last = (d == 0), (d == DT - 1)
                # long/short interleave so short-matmul weight loads hide
                for h in range(HPC):
                    nc.tensor.matmul(
                        pk[h],
                        lhsT=wk_sb[:, d, h * HD:(h + 1) * HD],
                        rhs=x_sb[:, d, csl(c)], start=first, stop=last,
                    )
                    for st in (0, 1) if h == 0 else (2, 3):
                        nc.tensor.matmul(
                            pv[st],
                            lhsT=x_sb[:, d, c * CH + st * P:c * CH + (st + 1) * P],
                            rhs=wv_sb[:, d, :],
                            start=first, stop=last,
                        )
            for h in range(HPC):
                nc.vector.tensor_copy(kT_sb[:, h, csl(c)], pk[h])
            for st in range(4):
                nc.vector.tensor_copy(v_sb[:, c * 4 + st, :], pv[st])

        kv_chunk(0)
        # chunk-0 q pass: re-reads resident data, buying the DMA stream slack
        # before chunk 1 is needed
        pq0 = [q_ps.tile([P, CH], f32, tag="pq", name=f"pq0_{i}") for i in range(HPC)]
        for d in range(DT):
            for h in range(HPC):
                nc.tensor.matmul(
                    pq0[h],
                    lhsT=wq_sb[:, d, h * HD:(h + 1) * HD],
                    rhs=x_sb[:, d, csl(0)], start=(d == 0), stop=(d == DT - 1),
                )
        for h in range(HPC):
            nc.vector.tensor_copy(qT_sb[:, h, csl(0)], pq0[h])
        kv_chunk(1)
        kv_chunk(2)
        kv_chunk(3)

        p1_ctx.close()  # release phase-1 PSUM banks

        # ---------------- attention pools ----------------
        sc_ps = ctx.enter_context(tc.tile_pool(name="scps", bufs=4, space="PSUM"))
        o_ps = ctx.enter_context(tc.tile_pool(name="ops", bufs=2, space="PSUM"))
        ro_ps = ctx.enter_context(tc.tile_pool(name="rops", bufs=2, space="PSUM"))
        pool_tag = {id(sc_ps): "psc", id(o_ps): "po", id(ro_ps): "rout"}

        # ---------------- filler emission units ----------------
        def make_defq_units(ci, h):
            """16 single-matmul units projecting q for attn chunk ci."""
            sl = asl(ci)
            w = ACH[ci][1]
            pq = ro_ps.tile([P, w], f32, tag="rout", name=f"dpq{ci}_{h}")

            def unit(d):
                def emit():
                    nc.tensor.matmul(
                        pq,
                        lhsT=wq_sb[:, d, h * HD:(h + 1) * HD],
                        rhs=x_sb[:, d, sl],
                        start=(d == 0), stop=(d == DT - 1),
                    )
                    if d == DT - 1:
                        nc.vector.tensor_copy(qT_sb[:, h, sl], pq)
                return emit

            return [unit(d) for d in range(DT)]

        def make_ph3_units(ci):
            sl = asl(ci)
            w = ACH[ci][1]

            def unit(ot):
                def emit():
                    pout = ro_ps.tile([P, w], f32, tag="rout", name=f"pout{ci}_{ot}")
                    for di in range(HDC // P):
                        nc.tensor.matmul(
                            pout,
                            lhsT=wo_sb[:, di, ot * P:(ot + 1) * P],
                            rhs=oT_sb[:, di, sl],
                            start=(di == 0), stop=(di == HDC // P - 1),
                        )
                    ob = obpool.tile([P, w], f16, tag="ob", name=f"ob{ci}_{ot}")
                    if ot % 4 == 3:
                        nc.scalar.copy(ob, pout)
                    else:
                        nc.vector.tensor_copy(ob, pout)
                    q = nc.sync if ot % 2 == 0 else nc.gpsimd
                    q.dma_start(out=out_r[ot][:, sl], in_=ob)
                return emit

            return [unit(ot) for ot in range(DT)]

        def emit_final_ph3():
            """Last chunk's output projection: pouts rotate across all free
            PSUM pools; ob tiles and out-DMAs are paired to halve the
            dispatch count on the critical drain."""
            ci = len(ACH) - 1
            sl = asl(ci)
            w = ACH[ci][1]
            out_p = outT.rearrange("(ot p) s -> p ot s", p=P)
            pools = [sc_ps, ro_ps, sc_ps, o_ps]
            for pair in range(DT // 2):
                ob2 = obpool.tile([P, 2, w], f16, tag="ob", name=f"obf{pair}")
                for j in range(2):
                    ot = pair * 2 + j
                    pool = pools[ot % 4]
                    pout = pool.tile([P, w], f32, tag=pool_tag[id(pool)],
                                     name=f"poutF_{ot}")
                    for di in range(HDC // P):
                        nc.tensor.matmul(
                            pout,
                            lhsT=wo_sb[:, di, ot * P:(ot + 1) * P],
                            rhs=oT_sb[:, di, sl],
                            start=(di == 0), stop=(di == HDC // P - 1),
                        )
                    if ot % 2 == 1:
                        nc.scalar.copy(ob2[:, j, :], pout)
                    else:
                        nc.vector.tensor_copy(ob2[:, j, :], pout)
                q = nc.sync if pair % 2 == 0 else nc.gpsimd
                q.dma_start(out=out_p[:, pair * 2:pair * 2 + 2, sl], in_=ob2)

        # ---------------- Phase 2: flat pipelined attention ----------------
        blocks = [(ci, h) for ci in range(len(ACH)) for h in range(HPC)]
        steps = [(bi, g) for bi in range(len(blocks)) for g in range(KT)]
        LOOKAHEAD = 3

        st_pt = {}      # (bi, g) -> pt AP for the PV matmul
        st_ptpair = {}  # bi -> current pt pair tile
        st_pscpair = {}  # bi -> current psum pair tile (narrow blocks)
        st_p0 = {}      # bi -> first pt pair (kept until the first dacc add)
        st_dacc2 = {}   # bi -> paired dacc accumulator
        st_dacc = {}    # bi -> folded dacc tile
        st_po = {}      # bi -> PV psum tile

        def emit_S(bi, g):
            ci, h = blocks[bi]
            w = ACH[ci][1]
            if g % 2 == 0:
                st_ptpair[bi] = ptpool.tile([P, 2, w], f16, tag="pt",
                                            name=f"ptp{bi}_{g // 2}")
            pt = st_ptpair[bi]
            if w == CH:
                psc = sc_ps.tile([P, w], f32, tag="psc", name=f"psc{bi}_{g}")
                nc.tensor.matmul(
                    psc,
                    lhsT=kT_sb[:, h, g * P:(g + 1) * P],
                    rhs=qT_sb[:, h, asl(ci)],
                    start=True, stop=True,
                )
                nc.scalar.activation(
                    out=pt[:, g % 2, :], in_=psc,
                    func=mybir.ActivationFunctionType.Exp, scale=SCALE,
                )
            else:
                # narrow blocks: two k-tiles share one psum tile and one exp
                if g % 2 == 0:
                    st_pscpair[bi] = sc_ps.tile([P, 2, w], f32, tag="psc",
                                                name=f"psc{bi}_{g}")
                psc = st_pscpair[bi]
                nc.tensor.matmul(
                    psc[:, g % 2, :],
                    lhsT=kT_sb[:, h, g * P:(g + 1) * P],
                    rhs=qT_sb[:, h, asl(ci)],
                    start=True, stop=True,
                )
                if g % 2 == 1:
                    nc.scalar.activation(
                        out=pt, in_=psc,
                        func=mybir.ActivationFunctionType.Exp, scale=SCALE,
                    )
            st_pt[(bi, g)] = pt[:, g % 2, :]
            # denominator accumulation on DVE over pt pairs (fp16, 2x mode)
            if g % 2 == 1:
                j = g // 2
                if j == 0:
                    st_p0[bi] = pt
                elif j == 1:
                    dacc2 = dapool.tile([P, 2, w], f16, tag="dacc", name=f"da{bi}")
                    st_dacc2[bi] = dacc2
                    nc.vector.tensor_add(dacc2, st_p0.pop(bi), pt)
                else:
                    nc.vector.tensor_add(st_dacc2[bi], st_dacc2[bi], pt)
                if g == KT - 1:
                    dacc = dapool.tile([P, w], f16, tag="dacc", name=f"daf{bi}")
                    st_dacc[bi] = dacc
                    nc.vector.tensor_add(
                        dacc, st_dacc2[bi][:, 0, :], st_dacc2[bi][:, 1, :])

        def emit_PV(bi, g):
            ci, h = blocks[bi]
            if g == 0:
                st_po[bi] = o_ps.tile([P, ACH[ci][1]], f32, tag="po", name=f"po{bi}")
            nc.tensor.matmul(
                st_po[bi],
                lhsT=v_sb[:, g, h * HD:(h + 1) * HD],
                rhs=st_pt.pop((bi, g)),
                start=(g == 0), stop=(g == KT - 1),
            )

        def emit_epilogue(bi):
            ci, h = blocks[bi]
            w = ACH[ci][1]
            pr = sc_ps.tile([P, w], f32, tag="psc", name=f"pr{bi}")
            nc.tensor.matmul(pr, lhsT=ones, rhs=st_dacc[bi], start=True, stop=True)
            rs = rspool.tile([P, w], f32, tag="rs", name=f"rs{bi}")
            nc.vector.reciprocal_approx_fast(out=rs, in_=pr)
            nc.vector.tensor_mul(oT_sb[:, h, asl(ci)], st_po[bi], rs)

        # per-block filler lists: next chunk's q projection, then previous
        # chunks' output projections (ph3(2) is split 5/3 so the short final
        # blocks have enough PE work to cover their exp latency)
        ph3u = {ci: make_ph3_units(ci) for ci in range(len(ACH) - 1)}
        fillers = []
        for bi, (ci, h) in enumerate(blocks):
            L = []
            if ci + 1 < len(ACH):
                L += make_defq_units(ci + 1, h)
            if ci >= 1:
                L += ph3u[ci - 1][h * 8:(h + 1) * 8]
            fillers.append(L)
        emitted = [0] * len(blocks)

        for t in range(LOOKAHEAD):
            emit_S(*steps[t])
        for t, (bi, g) in enumerate(steps):
            if g == 0 and bi > 0:
                emit_epilogue(bi - 1)
            L = fillers[bi]
            want = ((g + 1) * len(L) + KT - 1) // KT
            while emitted[bi] < min(want, len(L)):
                L[emitted[bi]]()
                emitted[bi] += 1
            emit_PV(bi, g)
            if t + LOOKAHEAD < len(steps):
                emit_S(*steps[t + LOOKAHEAD])
        emit_epilogue(len(blocks) - 1)
        emit_final_ph3()

    nc.compile()
    return nc


def _get_nc():
    if "nc" not in _CACHE:
        _CACHE["nc"] = _build_nc()
    return _CACHE["nc"]


def make_in_maps(x, w_q, w_k, w_v, w_o):
    x = np.asarray(x, dtype=np.float32).reshape(S, D)
    w_q = np.asarray(w_q, dtype=np.float32)
    w_k = np.asarray(w_k, dtype=np.float32)
    w_v = np.asarray(w_v, dtype=np.float32)
    w_o = np.asarray(w_o, dtype=np.float32)
    xT = np.ascontiguousarray(x.T).astype(np.float16)
    in_maps = []
    for c in range(NCORES):
        hs = slice(c * HDC, (c + 1) * HDC)
        in_maps.append({
            "xT": xT,
            "wqT": np.ascontiguousarray(w_q[hs, :].T).astype(np.float16),
            "wkT": np.ascontiguousarray(w_k[hs, :].T).astype(np.float16),
            "wvT": np.ascontiguousarray(w_v[hs, :].T).astype(np.float16),
            "woT": np.ascontiguousarray(w_o[:, hs].T).astype(np.float16),
        })
    return in_maps


def kernel(x, w_q, w_k, w_v, w_o):
    global LAST_RESULT
    in_maps = make_in_maps(x, w_q, w_k, w_v, w_o)
    nc = _get_nc()
    res = run_bass_kernel_spmd(nc, in_maps, core_ids=list(range(NCORES)))
    LAST_RESULT = res
    acc = np.zeros((D, S), dtype=np.float32)
    for r in res.results:
        acc += r["outT"].astype(np.float32)
    return np.ascontiguousarray(acc.T).astype(np.float32).reshape(1, S, D)
